# revision 62
# baseline (speedup 1.0000x reference)
"""Cross-attention + FFN + layernorm block on 8 Trainium2 NeuronCores.

Sharding: data-parallel over (B=4) x (LQ split in 2) -> 8 shards of 1024
query rows. Keys/values/weights are replicated per batch; each core runs
the full pipeline for its shard, so no collectives are needed.

v3 structure (PE-bound design, ~all other engines hidden under PE):
  - Prefix key mask -> only KC=ceil(max valid/128) key chunks computed;
    chunks < KF are valid for every batch (pairs share one exp call).
  - W_A = Wo@W1 folded on the host: h = relu(o @ W_A + b1). The residual
    o@Wo enters the W2 PSUM accumulation directly (no attT stage, no
    identity matmuls).
  - b2 is added during the W2 eviction (DVE tensor add with a broadcast
    tile), not via PE rank-1 matmuls.
  - o^T comes from the DMA XBAR (SBUF->SBUF), not PE transposes.
  - Layernorm rstd uses the magic-constant rsqrt on DVE (no ACT Sqrt, so
    the ACT table never swaps and LN runs immediately per row-block).
    The normalize+gamma apply is one fused custom-DVE op.
  - PSUM 'pos' banks are zeroed by the first AV matmul's start=True
    instead of DVE memsets.
  - Startup: per-chunk weight DMAs interleaved with XBAR x^T transposes
    in consumption order on the SP queue; FFN weights are fetched via
    the DVE queue once attention has started.
  - Attention is software-pipelined; projection/FFN chains drain between
    exp steps under a cost-weighted pacing so the PE never idles.
"""

import sys

if '/opt/trn_rl_repo' not in sys.path:
    sys.path.insert(0, '/opt/trn_rl_repo')

import numpy as np
import ml_dtypes

B, LQ, LK, D, H = 4, 2048, 2048, 768, 12
DH = D // H            # 64
NC = 8                 # cores
LQC = B * LQ // NC     # 1024 query rows per core
QB = LQC // 128        # 8 q row-tiles
C = D // 128           # 6 feature chunks
EPS = 1e-5

_CACHE = {}


def _build(KC, KF):
    """KC: number of 128-row key chunks kept; KF: chunks < KF are fully
    valid for every batch (no exp bias needed)."""
    import concourse.bacc as bacc
    import concourse.bass as bass
    import concourse.tile as tile
    import concourse.mybir as mybir

    f32 = mybir.dt.float32
    bf16 = mybir.dt.bfloat16
    i32 = mybir.dt.int32
    Exp = mybir.ActivationFunctionType.Exp
    Relu = mybir.ActivationFunctionType.Relu
    Copy = mybir.ActivationFunctionType.Copy
    Alu = mybir.AluOpType
    KCT = KC * 128

    nc = bacc.Bacc("TRN2", target_bir_lowering=False, debug=False)

    xq = nc.dram_tensor("xq", [LQC, D], bf16, kind="ExternalInput")
    xk = nc.dram_tensor("xk", [KCT, D], bf16, kind="ExternalInput")
    xv = nc.dram_tensor("xv", [KCT, D], bf16, kind="ExternalInput")
    # mbias columns 0:KC, b1 (per n-chunk) columns KC:KC+C
    mbias = nc.dram_tensor("mbias", [128, KC + C], f32, kind="ExternalInput")
    wq = nc.dram_tensor("wq", [D, D], bf16, kind="ExternalInput")
    wk = nc.dram_tensor("wk", [D, D], bf16, kind="ExternalInput")
    wv = nc.dram_tensor("wv", [D, D], bf16, kind="ExternalInput")
    wA = nc.dram_tensor("wA", [D, D], bf16, kind="ExternalInput")
    wo = nc.dram_tensor("wo", [D, D], bf16, kind="ExternalInput")
    w2 = nc.dram_tensor("w2", [D, D], bf16, kind="ExternalInput")
    b2v = nc.dram_tensor("b2v", [D], f32, kind="ExternalInput")
    gv = nc.dram_tensor("gv", [D], f32, kind="ExternalInput")
    bv = nc.dram_tensor("bv", [D], f32, kind="ExternalInput")
    yout = nc.dram_tensor("yout", [LQC, D], f32, kind="ExternalOutput")

    def w_ap(w):
        # [128(din part), C(din chunk), D(dout)] view of a [D, D] weight
        return w.ap().rearrange("(c p) n -> p c n", p=128)

    def bcast_ap(v):
        a = v.ap()
        return bass.AP(tensor=a.tensor, offset=a.offset, ap=[[0, 128]] + list(a.ap))

    # exp groups over key chunks: pairs among fully-valid chunks, singles
    # (with mask bias) for the rest
    groups = []
    kc = 0
    while kc + 1 < KF:
        groups.append(((kc, kc + 1), False))
        kc += 2
    if kc < KF:
        groups.append(((kc,), False))
        kc += 1
    while kc < KC:
        groups.append(((kc,), True))
        kc += 1
    NG = len(groups)

    with tile.TileContext(nc) as tc:
        with tc.tile_pool(name="consts", bufs=1) as consts, \
             tc.tile_pool(name="persist", bufs=1) as persist, \
             tc.tile_pool(name="work", bufs=2) as work, \
             tc.tile_pool(name="pp", bufs=2, space="PSUM") as pp:

            # ---- tiny int consts for the DVE rsqrt + identity for the
            # tail PE transposes
            shift1 = consts.tile([128, 1], i32)
            nc.gpsimd.memset(shift1, 1)
            magic = consts.tile([128, 1], i32)
            nc.gpsimd.memset(magic, 0x5F3759DF)
            from concourse.masks import make_identity
            ident = consts.tile([128, 128], bf16)
            make_identity(nc, ident)

            # ---- persistent activations (tags pair disjoint lifetimes)
            qT = persist.tile([128, C, LQC], bf16, tag="sQ")
            kT = persist.tile([128, C, KCT], bf16, tag="sK")
            vp = persist.tile([128, KC, H, DH + 1], bf16, tag="sV")
            xqT = persist.tile([128, C, LQC], bf16, tag="sA")
            xkT = persist.tile([128, C, KCT], bf16, tag="sB")
            xvT = persist.tile([128, C, KCT], bf16, tag="sC")
            o_sb = persist.tile([128, QB, D], bf16, tag="sO")
            wq_t = persist.tile([128, C, D], bf16, tag="wA")
            wk_t = persist.tile([128, C, D], bf16, tag="wB")
            wv_t = persist.tile([128, C, D], bf16, tag="wC")

            # denominator ones-column of vp (softmax sum via the o-matmul)
            nc.vector.memset(vp[:, :, :, DH:DH + 1], 1.0)

            # ---- input DMAs: few, big transfers in consumption order
            # (the DGE lane semaphores chain DMAs ~serially, so every
            # extra DMA costs ~1.9us of stream time).
            mbb = consts.tile([128, KC + C], f32)
            mb = mbb[:, 0:KC]
            b1_t = mbb[:, KC:KC + C]
            nc.sync.dma_start_transpose(xqT[:, :, :], xq.ap()[:, :])
            nc.sync.dma_start(out=wq_t, in_=w_ap(wq))
            nc.sync.dma_start_transpose(xkT[:, :, :], xk.ap()[:, :])
            nc.sync.dma_start(out=wk_t, in_=w_ap(wk))
            nc.sync.dma_start(out=mbb, in_=mbias.ap())
            nc.sync.dma_start_transpose(xvT[:, :, :], xv.ap()[:, :])
            nc.sync.dma_start(out=wv_t, in_=w_ap(wv))

            # FFN weights + broadcast consts: needed only from the FFN/LN
            # stages (~90us); hold their DMAs back (tile_wait_until informs
            # the scheduler; the SWDGE bcasts additionally get a WAW gate
            # via a tiny copy that depends on late data, since wait_until
            # is not honored on the Pool DMA path).
            wA_t = persist.tile([128, C, D], bf16, tag="wD")
            wo_t = persist.tile([128, C, D], bf16, tag="wE")
            w2_t = persist.tile([128, C, D], bf16, tag="wF")
            b2b = consts.tile([128, D], f32)
            g_t = consts.tile([128, D], f32)
            be_t = consts.tile([128, D], f32)
            with tc.tile_wait_until(0.048):
                nc.scalar.dma_start(out=wA_t, in_=w_ap(wA))
            with tc.tile_wait_until(0.052):
                nc.scalar.dma_start(out=wo_t, in_=w_ap(wo))
            with tc.tile_wait_until(0.056):
                nc.scalar.dma_start(out=w2_t, in_=w_ap(w2))
            for dst, src in ((b2b, b2v), (g_t, gv), (be_t, bv)):
                nc.gpsimd.tensor_copy(out=dst[0:1, 0:1],
                                      in_=vp[0:1, 0, 0, 0:1])
                nc.gpsimd.dma_start(out=dst, in_=bcast_ap(src))
            # oT reuses xkT's slot (K chains all consume xkT at startup);
            # hT reuses xqT's slot (Q chains done early in qc0 attention)
            oT = persist.tile([128, C, LQC], bf16, tag="sB")
            hT = persist.tile([128, C, LQC], bf16, tag="sA")
            # partial w1-qc1 accumulators (first 3 oT chunks, computed as
            # late-qc1 fillers; reloaded into PSUM at the tail)
            w1p = persist.tile([128, C, 512], bf16, tag="w1p")

            def evict(out_ap, in_ap, eng):
                if eng == "a":
                    nc.scalar.activation(out=out_ap, in_=in_ap, func=Copy,
                                         bias=0.0, scale=1.0)
                else:
                    nc.vector.tensor_copy(out=out_ap, in_=in_ap)

            # ---------------- chain builders ----------------
            def qk_chain(w_t, x_t, out_t, n, q0, qw, eng):
                def fn():
                    ps = pp.tile([128, 512], f32, tag="p512", name=f"ps_{n}_{q0}_{eng}")
                    for c in range(C):
                        nc.tensor.matmul(ps[:, 0:qw],
                                         w_t[:, c, n * 128:(n + 1) * 128],
                                         x_t[:, c, q0:q0 + qw],
                                         start=(c == 0), stop=(c == C - 1))
                    evict(out_t[:, n, q0:q0 + qw], ps[:, 0:qw], eng)
                return fn

            def v_chain(hp, kt):
                def fn():
                    ps = pp.tile([128, 512], f32, tag="p512", name=f"psv_{hp}_{kt}")
                    for c in range(C):
                        nc.tensor.matmul(ps[:, 0:128],
                                         xvT[:, c, kt * 128:(kt + 1) * 128],
                                         wv_t[:, c, hp * 128:(hp + 1) * 128],
                                         start=(c == 0), stop=(c == C - 1))
                    evict(vp[:, kt, 2 * hp:2 * hp + 2, 0:DH],
                          ps[:, 0:128].rearrange("p (h d) -> p h d", d=DH), "v")
                return fn

            def w1_chain(n, qc, eng):
                # hT[n,:] = relu(W_A[:,n]^T @ oT + b1[n]); two parts for
                # finer drain pacing
                st = {}

                def fa():
                    ps = pp.tile([128, 512], f32, tag="p512", name=f"ps1_{n}_{qc}")
                    st["ps"] = ps
                    for c in range(3):
                        nc.tensor.matmul(ps[:],
                                         wA_t[:, c, n * 128:(n + 1) * 128],
                                         oT[:, c, qc * 512:(qc + 1) * 512],
                                         start=(c == 0), stop=False)

                def fb():
                    ps = st["ps"]
                    for c in range(3, C):
                        nc.tensor.matmul(ps[:],
                                         wA_t[:, c, n * 128:(n + 1) * 128],
                                         oT[:, c, qc * 512:(qc + 1) * 512],
                                         start=False, stop=(c == C - 1))
                    if eng == "a":
                        nc.scalar.activation(
                            out=hT[:, n, qc * 512:(qc + 1) * 512], in_=ps[:],
                            func=Relu, bias=b1_t[:, n:n + 1], scale=1.0)
                    else:
                        nc.vector.tensor_scalar(
                            out=hT[:, n, qc * 512:(qc + 1) * 512], in0=ps[:],
                            scalar1=b1_t[:, n:n + 1], scalar2=0.0,
                            op0=Alu.add, op1=Alu.max)
                return fa, fb

            ysb_t = [None] * QB
            sums_t = [None] * QB
            Square = mybir.ActivationFunctionType.Square

            def w2_half(qb, half, act_stats=False):
                # ps = h^T@W2[:,half] + o^T@Wo[:,half]; evict adds b2.
                # Returns (partA, partB) so the drain pacing stays fine.
                n0, nw = (0, 512) if half == 0 else (512, 256)
                st = {}

                def fa():
                    ps = pp.tile([128, 512], f32, tag="p512",
                                 name=f"psy_{qb}_{half}")
                    st["ps"] = ps
                    for c in range(C):
                        nc.tensor.matmul(ps[:, 0:nw],
                                         hT[:, c, qb * 128:(qb + 1) * 128],
                                         w2_t[:, c, n0:n0 + nw],
                                         start=(c == 0), stop=False)

                def fb():
                    ps = st["ps"]
                    for c in range(C):
                        nc.tensor.matmul(ps[:, 0:nw],
                                         oT[:, c, qb * 128:(qb + 1) * 128],
                                         wo_t[:, c, n0:n0 + nw],
                                         start=False, stop=(c == C - 1),
                                         skip_group_check=True)
                    if half == 0:
                        ysb = work.tile([128, D], bf16, tag="ysb", bufs=4,
                                        name=f"y_{qb}")
                        ysb_t[qb] = ysb
                        sums_t[qb] = work.tile([128, 4], f32, tag="sums",
                                               bufs=3, name=f"sm_{qb}")
                    if act_stats:
                        # tail blocks: eviction + row-sum fused on DVE,
                        # sum-of-squares on the (idle) ACT engine writing
                        # its junk output back over the retired PSUM
                        sm = sums_t[qb]
                        nc.vector.tensor_tensor_reduce(
                            out=ysb_t[qb][:, n0:n0 + nw], in0=ps[:, 0:nw],
                            in1=b2b[:, n0:n0 + nw], scale=1.0, scalar=0.0,
                            op0=Alu.add, op1=Alu.add,
                            accum_out=sm[:, half:half + 1])
                        sq = work.tile([128, 512], bf16, tag="sqj", bufs=2,
                                       name=f"sq_{qb}_{half}")
                        nc.scalar.activation(
                            out=sq[:, 0:nw],
                            in_=ysb_t[qb][:, n0:n0 + nw],
                            func=Square,
                            accum_out=sm[:, 2 + half:3 + half])
                    else:
                        nc.vector.tensor_add(out=ysb_t[qb][:, n0:n0 + nw],
                                             in0=ps[:, 0:nw],
                                             in1=b2b[:, n0:n0 + nw])
                return fa, fb

            def stats_ln(qb, add_eng="p", use_sums=False, newtons=2,
                         stats_pre=None):
                # stats + DVE magic-rsqrt + fused normalize*gamma (+beta
                # on gpsimd or DVE) + output DMA on the ACT queue. No PE work.
                def fn():
                    ysb = ysb_t[qb]
                    mu = work.tile([128, 1], f32, tag="mu", bufs=3,
                                   name=f"mu_{qb}")
                    ve = work.tile([128, 1], f32, tag="ve", bufs=3,
                                   name=f"ve_{qb}")
                    if use_sums:
                        # mean/var from the fused row-sums: mu = s0/768,
                        # var = s2/768 - mu^2
                        sm = sums_t[qb]
                        t0 = work.tile([128, 2], f32, tag="t0", bufs=3,
                                       name=f"t0_{qb}")
                        nc.vector.tensor_add(out=t0[:, 0:1], in0=sm[:, 0:1],
                                             in1=sm[:, 1:2])
                        nc.vector.tensor_add(out=t0[:, 1:2], in0=sm[:, 2:3],
                                             in1=sm[:, 3:4])
                        nc.vector.tensor_scalar(out=mu, in0=t0[:, 0:1],
                                                scalar1=1.0 / D, scalar2=None,
                                                op0=Alu.mult)
                        mu2 = work.tile([128, 1], f32, tag="mu2", bufs=3,
                                        name=f"m2_{qb}")
                        nc.vector.tensor_mul(out=mu2, in0=mu, in1=mu)
                        nc.vector.tensor_scalar(out=ve, in0=t0[:, 1:2],
                                                scalar1=1.0 / D,
                                                scalar2=float(EPS),
                                                op0=Alu.mult, op1=Alu.add)
                        nc.vector.tensor_sub(out=ve, in0=ve, in1=mu2)
                    else:
                        if stats_pre is None:
                            stats = work.tile([128, 3, 6], f32, tag="stats",
                                              name=f"st_{qb}")
                            sgs = range(3)
                        else:
                            stats = stats_pre
                            sgs = (2,)
                        for sg in sgs:
                            nc.vector.bn_stats(
                                out=stats[:, sg, :],
                                in_=ysb[:, sg * 256:(sg + 1) * 256])
                        mv = work.tile([128, 2], f32, tag="mv", bufs=3,
                                       name=f"mv_{qb}")
                        nc.vector.bn_aggr(out=mv[:], in_=stats[:])
                        nc.vector.tensor_copy(out=mu, in_=mv[:, 0:1])
                        nc.vector.tensor_scalar(out=ve, in0=mv[:, 1:2],
                                                scalar1=float(EPS),
                                                scalar2=None, op0=Alu.add)
                    it = work.tile([128, 1], i32, tag="it", bufs=3,
                                   name=f"it_{qb}")
                    nc.vector.tensor_scalar(out=it, in0=ve.bitcast(i32),
                                            scalar1=shift1[:, 0:1],
                                            scalar2=None,
                                            op0=Alu.logical_shift_right)
                    nc.vector.tensor_sub(out=it, in0=magic, in1=it)
                    y = it.bitcast(f32)
                    t1 = work.tile([128, 1], f32, tag="t1", bufs=3,
                                   name=f"t1_{qb}")
                    for _ in range(newtons):
                        nc.vector.tensor_mul(out=t1, in0=y, in1=y)
                        nc.vector.tensor_mul(out=t1, in0=t1, in1=ve)
                        nc.vector.tensor_scalar(out=t1, in0=t1,
                                                scalar1=-0.5, scalar2=1.5,
                                                op0=Alu.mult, op1=Alu.add)
                        nc.vector.tensor_mul(out=y, in0=y, in1=t1)
                    nmu = work.tile([128, 1], f32, tag="nmu", bufs=3,
                                    name=f"nm_{qb}")
                    nc.vector.tensor_scalar(
                        out=nmu, in0=mu, scalar1=y[:, 0:1],
                        scalar2=-1.0, op0=Alu.mult, op1=Alu.mult)
                    yn = work.tile([128, D], f32, tag="yn", bufs=3,
                                   name=f"yn_{qb}")
                    if add_eng == "f":
                        # fused (y*rstd + nmu) + beta in one DVE op; exact
                        # for ln_g == 1 (true for this problem's inputs)
                        nc.vector.affine_then_add(out=yn, in0=ysb,
                                                  in1=be_t,
                                                  scale=y[:, 0:1],
                                                  bias=nmu[:, 0:1])
                    else:
                        acc = work.tile([128, 1], f32, tag="acc", bufs=3,
                                        name=f"ac_{qb}")
                        nc.vector.affine_mul_reduce(out=yn, accum_out=acc,
                                                    in0=ysb, in1=g_t,
                                                    scale=y[:, 0:1],
                                                    bias=nmu[:, 0:1])
                        if add_eng == "p":
                            nc.gpsimd.tensor_add(out=yn, in0=yn, in1=be_t)
                        else:
                            nc.vector.tensor_add(out=yn, in0=yn, in1=be_t)
                    nc.sync.dma_start(
                        out=yout.ap()[qb * 128:(qb + 1) * 128, :], in_=yn)
                return fn

            # ---------------- filler machinery ----------------
            # rounds: 2n = K-proj chains for chunk n (needed by head 2n
            # scores), 2n+1 = V chains for head-pair n (needed by its AV).
            # soft items have no deadline and are drained under pacing.
            rounds = {}
            softq = []
            spent = [0.0]
            total_cost = [0.0]

            def add_round(r, fn, cost):
                rounds.setdefault(r, []).append((fn, cost))
                total_cost[0] += cost

            def add_soft(fn, cost):
                softq.append((fn, cost))
                total_cost[0] += cost

            def force(r):
                for rr in sorted(k for k in rounds if k <= r):
                    for fn, cost in rounds.pop(rr):
                        fn()
                        spent[0] += cost

            def drain_next():
                # soft items first: round items are deadline-forced anyway,
                # and keeping them for their just-in-time force fills the
                # late-phase windows
                if softq:
                    fn, cost = softq.pop(0)
                elif rounds:
                    rr = min(rounds)
                    fn, cost = rounds[rr].pop(0)
                    if not rounds[rr]:
                        rounds.pop(rr)
                else:
                    return False
                fn()
                spent[0] += cost
                return True

            def drain_to(target):
                while spent[0] < target and drain_next():
                    pass

            # ---------------- attention ----------------
            def attn_phase(qc, post_head=None):
                seq = [(h, gi) for h in range(H) for gi in range(NG)]
                pos_t = {}
                pair_t = {}
                base = spent[0]
                phase_total = total_cost[0] - base
                nsteps = len(seq)

                def emit_scores(idx):
                    h, gi = seq[idx]
                    p0 = (h % 2) * 64
                    cc = h // 2
                    if gi == 0:
                        force(2 * cc + 1)  # kT chunk + V chains for pair
                        pos = pp.tile([128, 4, DH + 1], f32, tag="pos",
                                      name=f"pos_{qc}_{h}")
                        pos_t[h] = pos
                    g, _ = groups[gi]
                    ps_s = pp.tile([128, 2, 512], f32, tag="pair",
                                   name=f"pss_{qc}_{h}_{gi}")
                    for i, kc_ in enumerate(g):
                        nc.tensor.matmul(
                            ps_s[:, i, :],
                            kT[p0:p0 + 64, cc, kc_ * 128:(kc_ + 1) * 128],
                            qT[p0:p0 + 64, cc, qc * 512:(qc + 1) * 512],
                            start=True, stop=True)
                    pair_t[(h, gi)] = ps_s

                emit_scores(0)
                emit_scores(1)
                for idx, (h, gi) in enumerate(seq):
                    g, biased = groups[gi]
                    ps_s = pair_t.pop((h, gi))
                    ex = work.tile([128, 2, 512], bf16, tag="ex", bufs=4,
                                   name=f"ex_{qc}_{h}_{gi}")
                    glen = len(g)
                    bias = mb[:, g[0]:g[0] + 1] if biased else 0.0
                    nc.scalar.activation(out=ex[:, 0:glen, :],
                                         in_=ps_s[:, 0:glen, :], func=Exp,
                                         bias=bias, scale=1.0)
                    if idx + 2 < len(seq):
                        emit_scores(idx + 2)
                    if gi == 0:
                        force(2 * (h // 2) + 1)   # V chains for this pair
                    drain_to(base + phase_total * (idx + 1) / nsteps)
                    pos = pos_t[h]
                    first_av = (gi == 0)
                    for i, kc_ in enumerate(g):
                        for qs in range(4):
                            nc.tensor.matmul(
                                pos[:, qs, :],
                                ex[:, i, qs * 128:(qs + 1) * 128],
                                vp[:, kc_, h, :],
                                start=(first_av and i == 0 and qs == 0),
                                stop=(kc_ == KC - 1),
                                skip_group_check=not (first_av and i == 0
                                                      and qs == 0))
                    if gi == NG - 1:
                        # normalize -> o_sb
                        pos = pos_t.pop(h)
                        rec = work.tile([128, 4, 1], f32, tag="rec", bufs=3,
                                        name=f"rec_{qc}_{h}")
                        nc.vector.reciprocal(rec[:], pos[:, :, DH:DH + 1])
                        for qs in range(4):
                            qb = qc * 4 + qs
                            nc.vector.tensor_scalar_mul(
                                out=o_sb[:, qb, h * DH:(h + 1) * DH],
                                in0=pos[:, qs, 0:DH],
                                scalar1=rec[:, qs, 0:1])
                        if h % 2 == 1:
                            # o_sb chunk h//2 complete for this qc
                            cch = h // 2
                            if qc == 1 and h == H - 1:
                                # last chunk feeds the FFN tail right away:
                                # PE transpose + DVE copyback beats the
                                # XBAR round-trip latency here
                                for qs in range(4):
                                    qb = qc * 4 + qs
                                    ps = pp.tile([128, 512], f32, tag="p512",
                                                 name=f"pt_{qb}")
                                    pt = ps.bitcast(bf16)[:, 0:128]
                                    nc.tensor.transpose(
                                        pt,
                                        o_sb[:, qb, cch * 128:(cch + 1) * 128],
                                        ident[:])
                                    nc.vector.tensor_copy(
                                        out=oT[:, cch,
                                               qb * 128:(qb + 1) * 128],
                                        in_=pt)
                            else:
                                # XBAR transposes (SP queue) straight to oT
                                for qs in range(4):
                                    qb = qc * 4 + qs
                                    nc.sync.dma_start_transpose(
                                        oT[:, cch, qb * 128:(qb + 1) * 128],
                                        o_sb[:, qb,
                                             cch * 128:(cch + 1) * 128])
                        if post_head is not None:
                            post_head(h)

            # ---- startup PE stream, in DMA-arrival order: Q projections
            # (both halves), then all K chains. V chains stay as attention
            # fillers (rounds keyed to their consuming head pair).
            for n in range(C):
                qk_chain(wq_t, xqT, qT, n, 0, 512, "a")()
            kw = [(k0, min(512, KCT - k0)) for k0 in range(0, KCT, 512)]
            for n in range(C):
                for (k0, wdt) in kw:
                    qk_chain(wk_t, xkT, kT, n, k0, wdt, "v")()
            for n in range(C):
                for kt in range(KC):
                    add_round(2 * n + 1, v_chain(n, kt), 0.35)
            for n in range(C):
                add_soft(qk_chain(wq_t, xqT, qT, n, 512, 512, "v"), 1.28)

            # w1-qc0 first halves as late-qc0 fillers (oT-qc0 chunks 0-2
            # exist after qc0 head 5; round 7 gates them until head 6)
            def w1_fa_spill(n, qc):
                def fn():
                    ps = pp.tile([128, 512], f32, tag="p512",
                                 name=f"p1a_{n}_{qc}")
                    for c in range(3):
                        nc.tensor.matmul(ps[:],
                                         wA_t[:, c, n * 128:(n + 1) * 128],
                                         oT[:, c, qc * 512:(qc + 1) * 512],
                                         start=(c == 0), stop=(c == 2))
                    nc.vector.tensor_copy(out=w1p[:, n, :], in_=ps[:])
                return fn

            def w1_fb_reload(n, qc, eng):
                def fn():
                    ps = pp.tile([128, 512], f32, tag="p512",
                                 name=f"p1b_{n}_{qc}")
                    nc.tensor.matmul(ps[:], ident[:, :],
                                     w1p[:, n, :],
                                     start=True, stop=False)
                    for c in range(3, C):
                        nc.tensor.matmul(ps[:],
                                         wA_t[:, c, n * 128:(n + 1) * 128],
                                         oT[:, c, qc * 512:(qc + 1) * 512],
                                         start=False, stop=(c == C - 1),
                                         skip_group_check=True)
                    if eng == "a":
                        nc.scalar.activation(
                            out=hT[:, n, qc * 512:(qc + 1) * 512], in_=ps[:],
                            func=Relu, bias=b1_t[:, n:n + 1], scale=1.0)
                    else:
                        nc.vector.tensor_scalar(
                            out=hT[:, n, qc * 512:(qc + 1) * 512], in0=ps[:],
                            scalar1=b1_t[:, n:n + 1], scalar2=0.0,
                            op0=Alu.add, op1=Alu.max)
                return fn

            attn_phase(0)
            force(2 * C)

            # ---- qc1 fillers: qc0's FFN + layernorm pipeline
            for n in range(C):
                fa, fb = w1_chain(n, 0, "v")
                add_soft(fa, 0.64)
                add_soft(fb, 0.64)
            for qb in range(4):
                fa0, fb0 = w2_half(qb, 0)
                add_soft(fa0, 1.28)
                add_soft(fb0, 1.28)
                fa1, fb1 = w2_half(qb, 1)
                add_soft(fa1, 0.64)
                add_soft(fb1, 0.64)
                add_soft(stats_ln(qb), 0.05)

            attn_phase(1)
            while drain_next():
                pass

            # ---------------- qc1 FFN tail ----------------
            for n in range(C):
                fa, fb = w1_chain(n, 1, "a")
                fa()
                fb()
            for qb in (4, 5, 6, 7):
                fa0, fb0 = w2_half(qb, 0)
                fa1, fb1 = w2_half(qb, 1)
                fa0(); fb0()
                # stats for the first 512 columns overlap the second-half
                # matmuls on PE
                stp = work.tile([128, 3, 6], f32, tag="stats",
                                name=f"stp_{qb}")
                for sg in range(2):
                    nc.vector.bn_stats(
                        out=stp[:, sg, :],
                        in_=ysb_t[qb][:, sg * 256:(sg + 1) * 256])
                fa1(); fb1()
                stats_ln(qb, add_eng="p" if qb < 6 else "v",
                         stats_pre=stp, newtons=1 if qb == 7 else 2)()

    nc.compile()
    return nc


def _get_nc():
    # returns the most recently used compiled module (for test harness)
    key = _CACHE.get("last_key")
    if key is None:
        # default shape for this problem's input (valid_lens ~1028/1044/996)
        key = (9, 7)
    if ("nc", key) not in _CACHE:
        _CACHE[("nc", key)] = _build(*key)
    _CACHE["last_key"] = key
    return _CACHE[("nc", key)]


def _prepare_in_maps(queries, keys, values, mask, Wq, Wk, Wv, Wo, W1, b1,
                     W2, b2, ln_g, ln_b):
    bf16 = ml_dtypes.bfloat16
    queries = np.asarray(queries, dtype=np.float32)
    keys = np.asarray(keys, dtype=np.float32)
    values = np.asarray(values, dtype=np.float32)
    mask = np.asarray(mask)

    valid = (mask != 0).sum(axis=1).astype(np.int64)        # [B]
    valid = np.maximum(valid, 1)
    KC = int(-(-valid.max() // 128))
    KF = int(valid.min() // 128)
    KCT = KC * 128

    kidx = np.arange(KCT)
    mb_all = np.where(kidx[None, :] < valid[:, None], 0.0, -1e6)
    mb_all = mb_all.astype(np.float32).reshape(B, KC, 128).transpose(0, 2, 1)
    b1c = np.asarray(b1, np.float32).reshape(C, 128).T  # [128, C]

    wq_s = (np.asarray(Wq, np.float32) / np.sqrt(np.float32(DH)))
    wo_f = np.asarray(Wo, np.float32)
    w1_f = np.asarray(W1, np.float32)
    common = {
        "wq": wq_s.astype(bf16),
        "wk": np.asarray(Wk, np.float32).astype(bf16),
        "wv": np.asarray(Wv, np.float32).astype(bf16),
        "wA": (wo_f @ w1_f).astype(bf16),
        "wo": wo_f.astype(bf16),
        "w2": np.asarray(W2, np.float32).astype(bf16),
        "b2v": np.ascontiguousarray(np.asarray(b2, np.float32)),
        "gv": np.ascontiguousarray(ln_g, np.float32),
        "bv": np.ascontiguousarray(ln_b, np.float32),
    }

    in_maps = []
    for core in range(NC):
        b, half = core // 2, core % 2
        in_maps.append(dict(
            common,
            xq=np.ascontiguousarray(
                queries[b, half * LQC:(half + 1) * LQC, :]).astype(bf16),
            xk=np.ascontiguousarray(keys[b, :KCT, :]).astype(bf16),
            xv=np.ascontiguousarray(values[b, :KCT, :]).astype(bf16),
            mbias=np.ascontiguousarray(
                np.concatenate([mb_all[b], b1c], axis=1)),
        ))
    return in_maps, (KC, KF)


def kernel(queries, keys, values, mask, Wq, Wk, Wv, Wo, W1, b1, W2, b2,
           ln_g, ln_b, _trace=False):
    from concourse.bass_utils import run_bass_kernel_spmd

    in_maps, key = _prepare_in_maps(queries, keys, values, mask, Wq, Wk, Wv,
                                    Wo, W1, b1, W2, b2, ln_g, ln_b)
    if ("nc", key) not in _CACHE:
        _CACHE[("nc", key)] = _build(*key)
    _CACHE["last_key"] = key
    nc = _CACHE[("nc", key)]
    res = run_bass_kernel_spmd(nc, in_maps, core_ids=list(range(NC)),
                               trace=_trace)
    _CACHE["last_result"] = res

    out = np.empty((B, LQ, D), dtype=np.float32)
    for core in range(NC):
        b, half = core // 2, core % 2
        out[b, half * LQC:(half + 1) * LQC, :] = res.results[core]["yout"]
    return out


# revision 63
# speedup vs baseline: 1.0052x; 1.0052x over previous
"""Cross-attention + FFN + layernorm block on 8 Trainium2 NeuronCores.

Sharding: data-parallel over (B=4) x (LQ split in 2) -> 8 shards of 1024
query rows. Keys/values/weights are replicated per batch; each core runs
the full pipeline for its shard, so no collectives are needed.

v3 structure (PE-bound design, ~all other engines hidden under PE):
  - Prefix key mask -> only KC=ceil(max valid/128) key chunks computed;
    chunks < KF are valid for every batch (pairs share one exp call).
  - W_A = Wo@W1 folded on the host: h = relu(o @ W_A + b1). The residual
    o@Wo enters the W2 PSUM accumulation directly (no attT stage, no
    identity matmuls).
  - b2 is added during the W2 eviction (DVE tensor add with a broadcast
    tile), not via PE rank-1 matmuls.
  - o^T comes from the DMA XBAR (SBUF->SBUF), not PE transposes.
  - Layernorm rstd uses the magic-constant rsqrt on DVE (no ACT Sqrt, so
    the ACT table never swaps and LN runs immediately per row-block).
    The normalize+gamma apply is one fused custom-DVE op.
  - PSUM 'pos' banks are zeroed by the first AV matmul's start=True
    instead of DVE memsets.
  - Startup: per-chunk weight DMAs interleaved with XBAR x^T transposes
    in consumption order on the SP queue; FFN weights are fetched via
    the DVE queue once attention has started.
  - Attention is software-pipelined; projection/FFN chains drain between
    exp steps under a cost-weighted pacing so the PE never idles.
"""

import sys

if '/opt/trn_rl_repo' not in sys.path:
    sys.path.insert(0, '/opt/trn_rl_repo')

import numpy as np
import ml_dtypes

B, LQ, LK, D, H = 4, 2048, 2048, 768, 12
DH = D // H            # 64
NC = 8                 # cores
LQC = B * LQ // NC     # 1024 query rows per core
QB = LQC // 128        # 8 q row-tiles
C = D // 128           # 6 feature chunks
EPS = 1e-5

_CACHE = {}


def _build(KC, KF):
    """KC: number of 128-row key chunks kept; KF: chunks < KF are fully
    valid for every batch (no exp bias needed)."""
    import concourse.bacc as bacc
    import concourse.bass as bass
    import concourse.tile as tile
    import concourse.mybir as mybir

    f32 = mybir.dt.float32
    bf16 = mybir.dt.bfloat16
    i32 = mybir.dt.int32
    Exp = mybir.ActivationFunctionType.Exp
    Relu = mybir.ActivationFunctionType.Relu
    Copy = mybir.ActivationFunctionType.Copy
    Alu = mybir.AluOpType
    KCT = KC * 128

    nc = bacc.Bacc("TRN2", target_bir_lowering=False, debug=False)

    xq = nc.dram_tensor("xq", [LQC, D], bf16, kind="ExternalInput")
    xk = nc.dram_tensor("xk", [KCT, D], bf16, kind="ExternalInput")
    xv = nc.dram_tensor("xv", [KCT, D], bf16, kind="ExternalInput")
    # mbias columns 0:KC, b1 (per n-chunk) columns KC:KC+C
    mbias = nc.dram_tensor("mbias", [128, KC + C], f32, kind="ExternalInput")
    wq = nc.dram_tensor("wq", [D, D], bf16, kind="ExternalInput")
    wk = nc.dram_tensor("wk", [D, D], bf16, kind="ExternalInput")
    wv = nc.dram_tensor("wv", [D, D], bf16, kind="ExternalInput")
    wA = nc.dram_tensor("wA", [D, D], bf16, kind="ExternalInput")
    wo = nc.dram_tensor("wo", [D, D], bf16, kind="ExternalInput")
    w2 = nc.dram_tensor("w2", [D, D], bf16, kind="ExternalInput")
    b2v = nc.dram_tensor("b2v", [D], f32, kind="ExternalInput")
    gv = nc.dram_tensor("gv", [D], f32, kind="ExternalInput")
    bv = nc.dram_tensor("bv", [D], f32, kind="ExternalInput")
    yout = nc.dram_tensor("yout", [LQC, D], f32, kind="ExternalOutput")

    def w_ap(w):
        # [128(din part), C(din chunk), D(dout)] view of a [D, D] weight
        return w.ap().rearrange("(c p) n -> p c n", p=128)

    def bcast_ap(v):
        a = v.ap()
        return bass.AP(tensor=a.tensor, offset=a.offset, ap=[[0, 128]] + list(a.ap))

    # exp groups over key chunks: pairs among fully-valid chunks, singles
    # (with mask bias) for the rest
    groups = []
    kc = 0
    while kc + 1 < KF:
        groups.append(((kc, kc + 1), False))
        kc += 2
    if kc < KF:
        groups.append(((kc,), False))
        kc += 1
    while kc < KC:
        groups.append(((kc,), True))
        kc += 1
    NG = len(groups)

    with tile.TileContext(nc) as tc:
        with tc.tile_pool(name="consts", bufs=1) as consts, \
             tc.tile_pool(name="persist", bufs=1) as persist, \
             tc.tile_pool(name="work", bufs=2) as work, \
             tc.tile_pool(name="pp", bufs=2, space="PSUM") as pp:

            # ---- tiny int consts for the DVE rsqrt + identity for the
            # tail PE transposes
            shift1 = consts.tile([128, 1], i32)
            nc.gpsimd.memset(shift1, 1)
            magic = consts.tile([128, 1], i32)
            nc.gpsimd.memset(magic, 0x5F3759DF)
            from concourse.masks import make_identity
            ident = consts.tile([128, 128], bf16)
            make_identity(nc, ident)

            # ---- persistent activations (tags pair disjoint lifetimes)
            qT = persist.tile([128, C, LQC], bf16, tag="sQ")
            kT = persist.tile([128, C, KCT], bf16, tag="sK")
            vp = persist.tile([128, KC, H, DH + 1], bf16, tag="sV")
            xqT = persist.tile([128, C, LQC], bf16, tag="sA")
            xkT = persist.tile([128, C, KCT], bf16, tag="sB")
            xvT = persist.tile([128, C, KCT], bf16, tag="sC")
            o_sb = persist.tile([128, QB, D], bf16, tag="sO")
            wq_t = persist.tile([128, C, D], bf16, tag="wA")
            wk_t = persist.tile([128, C, D], bf16, tag="wB")
            wv_t = persist.tile([128, C, D], bf16, tag="wC")

            # denominator ones-column of vp (softmax sum via the o-matmul)
            nc.vector.memset(vp[:, :, :, DH:DH + 1], 1.0)

            # ---- input DMAs: few, big transfers in consumption order
            # (the DGE lane semaphores chain DMAs ~serially, so every
            # extra DMA costs ~1.9us of stream time).
            mbb = consts.tile([128, KC + C], f32)
            mb = mbb[:, 0:KC]
            b1_t = mbb[:, KC:KC + C]
            nc.sync.dma_start_transpose(xqT[:, :, :], xq.ap()[:, :])
            nc.sync.dma_start(out=wq_t, in_=w_ap(wq))
            nc.sync.dma_start_transpose(xkT[:, :, :], xk.ap()[:, :])
            nc.sync.dma_start(out=wk_t, in_=w_ap(wk))
            nc.sync.dma_start(out=mbb, in_=mbias.ap())
            nc.sync.dma_start_transpose(xvT[:, :, :], xv.ap()[:, :])
            nc.sync.dma_start(out=wv_t, in_=w_ap(wv))

            # FFN weights + broadcast consts: needed only from the FFN/LN
            # stages (~90us); hold their DMAs back (tile_wait_until informs
            # the scheduler; the SWDGE bcasts additionally get a WAW gate
            # via a tiny copy that depends on late data, since wait_until
            # is not honored on the Pool DMA path).
            wA_t = persist.tile([128, C, D], bf16, tag="wD")
            wo_t = persist.tile([128, C, D], bf16, tag="wE")
            w2_t = persist.tile([128, C, D], bf16, tag="wF")
            b2b = consts.tile([128, D], f32)
            g_t = consts.tile([128, D], f32)
            be_t = consts.tile([128, D], f32)
            with tc.tile_wait_until(0.048):
                nc.scalar.dma_start(out=wA_t, in_=w_ap(wA))
            with tc.tile_wait_until(0.052):
                nc.scalar.dma_start(out=wo_t, in_=w_ap(wo))
            with tc.tile_wait_until(0.056):
                nc.scalar.dma_start(out=w2_t, in_=w_ap(w2))
            for dst, src in ((b2b, b2v), (g_t, gv), (be_t, bv)):
                nc.gpsimd.tensor_copy(out=dst[0:1, 0:1],
                                      in_=vp[0:1, 0, 0, 0:1])
                nc.gpsimd.dma_start(out=dst, in_=bcast_ap(src))
            # oT reuses xkT's slot (K chains all consume xkT at startup);
            # hT reuses xqT's slot (Q chains done early in qc0 attention)
            oT = persist.tile([128, C, LQC], bf16, tag="sB")
            hT = persist.tile([128, C, LQC], bf16, tag="sA")
            # partial w1-qc1 accumulators (first 3 oT chunks, computed as
            # late-qc1 fillers; reloaded into PSUM at the tail)
            w1p = persist.tile([128, C, 512], bf16, tag="w1p")

            def evict(out_ap, in_ap, eng):
                if eng == "a":
                    nc.scalar.activation(out=out_ap, in_=in_ap, func=Copy,
                                         bias=0.0, scale=1.0)
                else:
                    nc.vector.tensor_copy(out=out_ap, in_=in_ap)

            # ---------------- chain builders ----------------
            def qk_chain(w_t, x_t, out_t, n, q0, qw, eng):
                def fn():
                    ps = pp.tile([128, 512], f32, tag="p512", name=f"ps_{n}_{q0}_{eng}")
                    for c in range(C):
                        nc.tensor.matmul(ps[:, 0:qw],
                                         w_t[:, c, n * 128:(n + 1) * 128],
                                         x_t[:, c, q0:q0 + qw],
                                         start=(c == 0), stop=(c == C - 1))
                    evict(out_t[:, n, q0:q0 + qw], ps[:, 0:qw], eng)
                return fn

            def v_chain(hp, kt):
                def fn():
                    ps = pp.tile([128, 512], f32, tag="p512", name=f"psv_{hp}_{kt}")
                    for c in range(C):
                        nc.tensor.matmul(ps[:, 0:128],
                                         xvT[:, c, kt * 128:(kt + 1) * 128],
                                         wv_t[:, c, hp * 128:(hp + 1) * 128],
                                         start=(c == 0), stop=(c == C - 1))
                    evict(vp[:, kt, 2 * hp:2 * hp + 2, 0:DH],
                          ps[:, 0:128].rearrange("p (h d) -> p h d", d=DH), "v")
                return fn

            def w1_chain(n, qc, eng):
                # hT[n,:] = relu(W_A[:,n]^T @ oT + b1[n]); two parts for
                # finer drain pacing
                st = {}

                def fa():
                    ps = pp.tile([128, 512], f32, tag="p512", name=f"ps1_{n}_{qc}")
                    st["ps"] = ps
                    for c in range(3):
                        nc.tensor.matmul(ps[:],
                                         wA_t[:, c, n * 128:(n + 1) * 128],
                                         oT[:, c, qc * 512:(qc + 1) * 512],
                                         start=(c == 0), stop=False)

                def fb():
                    ps = st["ps"]
                    for c in range(3, C):
                        nc.tensor.matmul(ps[:],
                                         wA_t[:, c, n * 128:(n + 1) * 128],
                                         oT[:, c, qc * 512:(qc + 1) * 512],
                                         start=False, stop=(c == C - 1))
                    if eng == "a":
                        nc.scalar.activation(
                            out=hT[:, n, qc * 512:(qc + 1) * 512], in_=ps[:],
                            func=Relu, bias=b1_t[:, n:n + 1], scale=1.0)
                    else:
                        nc.vector.tensor_scalar(
                            out=hT[:, n, qc * 512:(qc + 1) * 512], in0=ps[:],
                            scalar1=b1_t[:, n:n + 1], scalar2=0.0,
                            op0=Alu.add, op1=Alu.max)
                return fa, fb

            ysb_t = [None] * QB
            sums_t = [None] * QB
            Square = mybir.ActivationFunctionType.Square

            def w2_half(qb, half, act_stats=False):
                # ps = h^T@W2[:,half] + o^T@Wo[:,half]; evict adds b2.
                # Returns (partA, partB) so the drain pacing stays fine.
                n0, nw = (0, 512) if half == 0 else (512, 256)
                st = {}

                def fa():
                    ps = pp.tile([128, 512], f32, tag="p512",
                                 name=f"psy_{qb}_{half}")
                    st["ps"] = ps
                    for c in range(C):
                        nc.tensor.matmul(ps[:, 0:nw],
                                         hT[:, c, qb * 128:(qb + 1) * 128],
                                         w2_t[:, c, n0:n0 + nw],
                                         start=(c == 0), stop=False)

                def fb():
                    ps = st["ps"]
                    for c in range(C):
                        nc.tensor.matmul(ps[:, 0:nw],
                                         oT[:, c, qb * 128:(qb + 1) * 128],
                                         wo_t[:, c, n0:n0 + nw],
                                         start=False, stop=(c == C - 1),
                                         skip_group_check=True)
                    if half == 0:
                        ysb = work.tile([128, D], bf16, tag="ysb", bufs=4,
                                        name=f"y_{qb}")
                        ysb_t[qb] = ysb
                        sums_t[qb] = work.tile([128, 4], f32, tag="sums",
                                               bufs=3, name=f"sm_{qb}")
                    if act_stats:
                        # tail blocks: eviction + row-sum fused on DVE,
                        # sum-of-squares on the (idle) ACT engine writing
                        # its junk output back over the retired PSUM
                        sm = sums_t[qb]
                        nc.vector.tensor_tensor_reduce(
                            out=ysb_t[qb][:, n0:n0 + nw], in0=ps[:, 0:nw],
                            in1=b2b[:, n0:n0 + nw], scale=1.0, scalar=0.0,
                            op0=Alu.add, op1=Alu.add,
                            accum_out=sm[:, half:half + 1])
                        sq = work.tile([128, 512], bf16, tag="sqj", bufs=2,
                                       name=f"sq_{qb}_{half}")
                        nc.scalar.activation(
                            out=sq[:, 0:nw],
                            in_=ysb_t[qb][:, n0:n0 + nw],
                            func=Square,
                            accum_out=sm[:, 2 + half:3 + half])
                    else:
                        nc.vector.tensor_add(out=ysb_t[qb][:, n0:n0 + nw],
                                             in0=ps[:, 0:nw],
                                             in1=b2b[:, n0:n0 + nw])
                return fa, fb

            def stats_ln(qb, add_eng="p", use_sums=False, newtons=2,
                         stats_pre=None):
                # stats + DVE magic-rsqrt + fused normalize*gamma (+beta
                # on gpsimd or DVE) + output DMA on the ACT queue. No PE work.
                def fn():
                    ysb = ysb_t[qb]
                    mu = work.tile([128, 1], f32, tag="mu", bufs=3,
                                   name=f"mu_{qb}")
                    ve = work.tile([128, 1], f32, tag="ve", bufs=3,
                                   name=f"ve_{qb}")
                    if use_sums:
                        # mean/var from the fused row-sums: mu = s0/768,
                        # var = s2/768 - mu^2
                        sm = sums_t[qb]
                        t0 = work.tile([128, 2], f32, tag="t0", bufs=3,
                                       name=f"t0_{qb}")
                        nc.vector.tensor_add(out=t0[:, 0:1], in0=sm[:, 0:1],
                                             in1=sm[:, 1:2])
                        nc.vector.tensor_add(out=t0[:, 1:2], in0=sm[:, 2:3],
                                             in1=sm[:, 3:4])
                        nc.vector.tensor_scalar(out=mu, in0=t0[:, 0:1],
                                                scalar1=1.0 / D, scalar2=None,
                                                op0=Alu.mult)
                        mu2 = work.tile([128, 1], f32, tag="mu2", bufs=3,
                                        name=f"m2_{qb}")
                        nc.vector.tensor_mul(out=mu2, in0=mu, in1=mu)
                        nc.vector.tensor_scalar(out=ve, in0=t0[:, 1:2],
                                                scalar1=1.0 / D,
                                                scalar2=float(EPS),
                                                op0=Alu.mult, op1=Alu.add)
                        nc.vector.tensor_sub(out=ve, in0=ve, in1=mu2)
                    else:
                        if stats_pre is None:
                            stats = work.tile([128, 3, 6], f32, tag="stats",
                                              name=f"st_{qb}")
                            sgs = range(3)
                        else:
                            stats = stats_pre
                            sgs = (2,)
                        for sg in sgs:
                            nc.vector.bn_stats(
                                out=stats[:, sg, :],
                                in_=ysb[:, sg * 256:(sg + 1) * 256])
                        mv = work.tile([128, 2], f32, tag="mv", bufs=3,
                                       name=f"mv_{qb}")
                        nc.vector.bn_aggr(out=mv[:], in_=stats[:])
                        nc.vector.tensor_copy(out=mu, in_=mv[:, 0:1])
                        nc.vector.tensor_scalar(out=ve, in0=mv[:, 1:2],
                                                scalar1=float(EPS),
                                                scalar2=None, op0=Alu.add)
                    it = work.tile([128, 1], i32, tag="it", bufs=3,
                                   name=f"it_{qb}")
                    nc.vector.tensor_scalar(out=it, in0=ve.bitcast(i32),
                                            scalar1=shift1[:, 0:1],
                                            scalar2=None,
                                            op0=Alu.logical_shift_right)
                    nc.vector.tensor_sub(out=it, in0=magic, in1=it)
                    y = it.bitcast(f32)
                    t1 = work.tile([128, 1], f32, tag="t1", bufs=3,
                                   name=f"t1_{qb}")
                    for _ in range(newtons):
                        nc.vector.tensor_mul(out=t1, in0=y, in1=y)
                        nc.vector.tensor_mul(out=t1, in0=t1, in1=ve)
                        nc.vector.tensor_scalar(out=t1, in0=t1,
                                                scalar1=-0.5, scalar2=1.5,
                                                op0=Alu.mult, op1=Alu.add)
                        nc.vector.tensor_mul(out=y, in0=y, in1=t1)
                    nmu = work.tile([128, 1], f32, tag="nmu", bufs=3,
                                    name=f"nm_{qb}")
                    nc.vector.tensor_scalar(
                        out=nmu, in0=mu, scalar1=y[:, 0:1],
                        scalar2=-1.0, op0=Alu.mult, op1=Alu.mult)
                    yn = work.tile([128, D], f32, tag="yn", bufs=3,
                                   name=f"yn_{qb}")
                    if add_eng == "f":
                        # fused (y*rstd + nmu) + beta in one DVE op; exact
                        # for ln_g == 1 (true for this problem's inputs)
                        nc.vector.affine_then_add(out=yn, in0=ysb,
                                                  in1=be_t,
                                                  scale=y[:, 0:1],
                                                  bias=nmu[:, 0:1])
                    else:
                        acc = work.tile([128, 1], f32, tag="acc", bufs=3,
                                        name=f"ac_{qb}")
                        nc.vector.affine_mul_reduce(out=yn, accum_out=acc,
                                                    in0=ysb, in1=g_t,
                                                    scale=y[:, 0:1],
                                                    bias=nmu[:, 0:1])
                        if add_eng == "p":
                            nc.gpsimd.tensor_add(out=yn, in0=yn, in1=be_t)
                        else:
                            nc.vector.tensor_add(out=yn, in0=yn, in1=be_t)
                    nc.sync.dma_start(
                        out=yout.ap()[qb * 128:(qb + 1) * 128, :], in_=yn)
                return fn

            # ---------------- filler machinery ----------------
            # rounds: 2n = K-proj chains for chunk n (needed by head 2n
            # scores), 2n+1 = V chains for head-pair n (needed by its AV).
            # soft items have no deadline and are drained under pacing.
            rounds = {}
            softq = []
            spent = [0.0]
            total_cost = [0.0]

            def add_round(r, fn, cost):
                rounds.setdefault(r, []).append((fn, cost))
                total_cost[0] += cost

            def add_soft(fn, cost):
                softq.append((fn, cost))
                total_cost[0] += cost

            def force(r):
                for rr in sorted(k for k in rounds if k <= r):
                    for fn, cost in rounds.pop(rr):
                        fn()
                        spent[0] += cost

            def drain_next():
                # soft items first: round items are deadline-forced anyway,
                # and keeping them for their just-in-time force fills the
                # late-phase windows
                if softq:
                    fn, cost = softq.pop(0)
                elif rounds:
                    rr = min(rounds)
                    fn, cost = rounds[rr].pop(0)
                    if not rounds[rr]:
                        rounds.pop(rr)
                else:
                    return False
                fn()
                spent[0] += cost
                return True

            def drain_to(target):
                while spent[0] < target and drain_next():
                    pass

            # ---------------- attention ----------------
            def attn_phase(qc, post_head=None):
                seq = [(h, gi) for h in range(H) for gi in range(NG)]
                pos_t = {}
                pair_t = {}
                base = spent[0]
                phase_total = total_cost[0] - base
                # pace by cumulative exp-engine time, not step count
                wts = [1038.0 if len(groups[gi][0]) == 2 else 612.0
                       for (_h, gi) in seq]
                cumw = []
                acc = 0.0
                for w in wts:
                    acc += w
                    cumw.append(acc)
                wtot = acc

                def emit_scores(idx):
                    h, gi = seq[idx]
                    p0 = (h % 2) * 64
                    cc = h // 2
                    if gi == 0:
                        force(2 * cc + 1)  # kT chunk + V chains for pair
                        pos = pp.tile([128, 4, DH + 1], f32, tag="pos",
                                      name=f"pos_{qc}_{h}")
                        pos_t[h] = pos
                    g, _ = groups[gi]
                    ps_s = pp.tile([128, 2, 512], f32, tag="pair",
                                   name=f"pss_{qc}_{h}_{gi}")
                    for i, kc_ in enumerate(g):
                        nc.tensor.matmul(
                            ps_s[:, i, :],
                            kT[p0:p0 + 64, cc, kc_ * 128:(kc_ + 1) * 128],
                            qT[p0:p0 + 64, cc, qc * 512:(qc + 1) * 512],
                            start=True, stop=True)
                    pair_t[(h, gi)] = ps_s

                emit_scores(0)
                emit_scores(1)
                for idx, (h, gi) in enumerate(seq):
                    g, biased = groups[gi]
                    ps_s = pair_t.pop((h, gi))
                    ex = work.tile([128, 2, 512], bf16, tag="ex", bufs=6,
                                   name=f"ex_{qc}_{h}_{gi}")
                    glen = len(g)
                    bias = mb[:, g[0]:g[0] + 1] if biased else 0.0
                    nc.scalar.activation(out=ex[:, 0:glen, :],
                                         in_=ps_s[:, 0:glen, :], func=Exp,
                                         bias=bias, scale=1.0)
                    if idx + 2 < len(seq):
                        emit_scores(idx + 2)
                    if gi == 0:
                        force(2 * (h // 2) + 1)   # V chains for this pair
                    drain_to(base + phase_total * cumw[idx] / wtot)
                    pos = pos_t[h]
                    first_av = (gi == 0)
                    for i, kc_ in enumerate(g):
                        for qs in range(4):
                            nc.tensor.matmul(
                                pos[:, qs, :],
                                ex[:, i, qs * 128:(qs + 1) * 128],
                                vp[:, kc_, h, :],
                                start=(first_av and i == 0 and qs == 0),
                                stop=(kc_ == KC - 1),
                                skip_group_check=not (first_av and i == 0
                                                      and qs == 0))
                    if gi == NG - 1:
                        # normalize -> o_sb
                        pos = pos_t.pop(h)
                        rec = work.tile([128, 4, 1], f32, tag="rec", bufs=3,
                                        name=f"rec_{qc}_{h}")
                        nc.vector.reciprocal(rec[:], pos[:, :, DH:DH + 1])
                        for qs in range(4):
                            qb = qc * 4 + qs
                            nc.vector.tensor_scalar_mul(
                                out=o_sb[:, qb, h * DH:(h + 1) * DH],
                                in0=pos[:, qs, 0:DH],
                                scalar1=rec[:, qs, 0:1])
                        if h % 2 == 1:
                            # o_sb chunk h//2 complete for this qc
                            cch = h // 2
                            if qc == 1 and h == H - 1:
                                # last chunk feeds the FFN tail right away:
                                # PE transpose + DVE copyback beats the
                                # XBAR round-trip latency here
                                for qs in range(4):
                                    qb = qc * 4 + qs
                                    ps = pp.tile([128, 512], f32, tag="p512",
                                                 name=f"pt_{qb}")
                                    pt = ps.bitcast(bf16)[:, 0:128]
                                    nc.tensor.transpose(
                                        pt,
                                        o_sb[:, qb, cch * 128:(cch + 1) * 128],
                                        ident[:])
                                    nc.vector.tensor_copy(
                                        out=oT[:, cch,
                                               qb * 128:(qb + 1) * 128],
                                        in_=pt)
                            else:
                                # XBAR transposes (SP queue) straight to oT
                                for qs in range(4):
                                    qb = qc * 4 + qs
                                    nc.sync.dma_start_transpose(
                                        oT[:, cch, qb * 128:(qb + 1) * 128],
                                        o_sb[:, qb,
                                             cch * 128:(cch + 1) * 128])
                        if post_head is not None:
                            post_head(h)

            # ---- startup PE stream, in DMA-arrival order: Q projections
            # (both halves), then all K chains. V chains stay as attention
            # fillers (rounds keyed to their consuming head pair).
            for n in range(C):
                qk_chain(wq_t, xqT, qT, n, 0, 512, "a")()
            kw = [(k0, min(512, KCT - k0)) for k0 in range(0, KCT, 512)]
            for n in range(C):
                for (k0, wdt) in kw:
                    qk_chain(wk_t, xkT, kT, n, k0, wdt, "v")()
            for n in range(C):
                for kt in range(KC):
                    add_round(2 * n + 1, v_chain(n, kt), 0.35)
            for n in range(C):
                add_soft(qk_chain(wq_t, xqT, qT, n, 512, 512, "v"), 1.28)

            # w1-qc0 first halves as late-qc0 fillers (oT-qc0 chunks 0-2
            # exist after qc0 head 5; round 7 gates them until head 6)
            def w1_fa_spill(n, qc):
                def fn():
                    ps = pp.tile([128, 512], f32, tag="p512",
                                 name=f"p1a_{n}_{qc}")
                    for c in range(3):
                        nc.tensor.matmul(ps[:],
                                         wA_t[:, c, n * 128:(n + 1) * 128],
                                         oT[:, c, qc * 512:(qc + 1) * 512],
                                         start=(c == 0), stop=(c == 2))
                    nc.vector.tensor_copy(out=w1p[:, n, :], in_=ps[:])
                return fn

            def w1_fb_reload(n, qc, eng):
                def fn():
                    ps = pp.tile([128, 512], f32, tag="p512",
                                 name=f"p1b_{n}_{qc}")
                    nc.tensor.matmul(ps[:], ident[:, :],
                                     w1p[:, n, :],
                                     start=True, stop=False)
                    for c in range(3, C):
                        nc.tensor.matmul(ps[:],
                                         wA_t[:, c, n * 128:(n + 1) * 128],
                                         oT[:, c, qc * 512:(qc + 1) * 512],
                                         start=False, stop=(c == C - 1),
                                         skip_group_check=True)
                    if eng == "a":
                        nc.scalar.activation(
                            out=hT[:, n, qc * 512:(qc + 1) * 512], in_=ps[:],
                            func=Relu, bias=b1_t[:, n:n + 1], scale=1.0)
                    else:
                        nc.vector.tensor_scalar(
                            out=hT[:, n, qc * 512:(qc + 1) * 512], in0=ps[:],
                            scalar1=b1_t[:, n:n + 1], scalar2=0.0,
                            op0=Alu.add, op1=Alu.max)
                return fn

            attn_phase(0)
            force(2 * C)

            # ---- qc1 fillers: qc0's FFN + layernorm pipeline
            for n in range(C):
                fa, fb = w1_chain(n, 0, "v")
                add_soft(fa, 0.64)
                add_soft(fb, 0.64)
            for qb in range(4):
                fa0, fb0 = w2_half(qb, 0)
                add_soft(fa0, 1.28)
                add_soft(fb0, 1.28)
                fa1, fb1 = w2_half(qb, 1)
                add_soft(fa1, 0.64)
                add_soft(fb1, 0.64)
                add_soft(stats_ln(qb), 0.05)

            attn_phase(1)
            while drain_next():
                pass

            # ---------------- qc1 FFN tail ----------------
            for n in range(C):
                fa, fb = w1_chain(n, 1, "a")
                fa()
                fb()
            for qb in (4, 5, 6, 7):
                fa0, fb0 = w2_half(qb, 0)
                fa1, fb1 = w2_half(qb, 1)
                fa0(); fb0()
                # stats for the first 512 columns overlap the second-half
                # matmuls on PE
                stp = work.tile([128, 3, 6], f32, tag="stats",
                                name=f"stp_{qb}")
                for sg in range(2):
                    nc.vector.bn_stats(
                        out=stp[:, sg, :],
                        in_=ysb_t[qb][:, sg * 256:(sg + 1) * 256])
                fa1(); fb1()
                stats_ln(qb, add_eng="p" if qb < 6 else "v",
                         stats_pre=stp, newtons=1 if qb == 7 else 2)()

    nc.compile()
    return nc


def _get_nc():
    # returns the most recently used compiled module (for test harness)
    key = _CACHE.get("last_key")
    if key is None:
        # default shape for this problem's input (valid_lens ~1028/1044/996)
        key = (9, 7)
    if ("nc", key) not in _CACHE:
        _CACHE[("nc", key)] = _build(*key)
    _CACHE["last_key"] = key
    return _CACHE[("nc", key)]


def _prepare_in_maps(queries, keys, values, mask, Wq, Wk, Wv, Wo, W1, b1,
                     W2, b2, ln_g, ln_b):
    bf16 = ml_dtypes.bfloat16
    queries = np.asarray(queries, dtype=np.float32)
    keys = np.asarray(keys, dtype=np.float32)
    values = np.asarray(values, dtype=np.float32)
    mask = np.asarray(mask)

    valid = (mask != 0).sum(axis=1).astype(np.int64)        # [B]
    valid = np.maximum(valid, 1)
    KC = int(-(-valid.max() // 128))
    KF = int(valid.min() // 128)
    KCT = KC * 128

    kidx = np.arange(KCT)
    mb_all = np.where(kidx[None, :] < valid[:, None], 0.0, -1e6)
    mb_all = mb_all.astype(np.float32).reshape(B, KC, 128).transpose(0, 2, 1)
    b1c = np.asarray(b1, np.float32).reshape(C, 128).T  # [128, C]

    wq_s = (np.asarray(Wq, np.float32) / np.sqrt(np.float32(DH)))
    wo_f = np.asarray(Wo, np.float32)
    w1_f = np.asarray(W1, np.float32)
    common = {
        "wq": wq_s.astype(bf16),
        "wk": np.asarray(Wk, np.float32).astype(bf16),
        "wv": np.asarray(Wv, np.float32).astype(bf16),
        "wA": (wo_f @ w1_f).astype(bf16),
        "wo": wo_f.astype(bf16),
        "w2": np.asarray(W2, np.float32).astype(bf16),
        "b2v": np.ascontiguousarray(np.asarray(b2, np.float32)),
        "gv": np.ascontiguousarray(ln_g, np.float32),
        "bv": np.ascontiguousarray(ln_b, np.float32),
    }

    in_maps = []
    for core in range(NC):
        b, half = core // 2, core % 2
        in_maps.append(dict(
            common,
            xq=np.ascontiguousarray(
                queries[b, half * LQC:(half + 1) * LQC, :]).astype(bf16),
            xk=np.ascontiguousarray(keys[b, :KCT, :]).astype(bf16),
            xv=np.ascontiguousarray(values[b, :KCT, :]).astype(bf16),
            mbias=np.ascontiguousarray(
                np.concatenate([mb_all[b], b1c], axis=1)),
        ))
    return in_maps, (KC, KF)


def kernel(queries, keys, values, mask, Wq, Wk, Wv, Wo, W1, b1, W2, b2,
           ln_g, ln_b, _trace=False):
    from concourse.bass_utils import run_bass_kernel_spmd

    in_maps, key = _prepare_in_maps(queries, keys, values, mask, Wq, Wk, Wv,
                                    Wo, W1, b1, W2, b2, ln_g, ln_b)
    if ("nc", key) not in _CACHE:
        _CACHE[("nc", key)] = _build(*key)
    _CACHE["last_key"] = key
    nc = _CACHE[("nc", key)]
    res = run_bass_kernel_spmd(nc, in_maps, core_ids=list(range(NC)),
                               trace=_trace)
    _CACHE["last_result"] = res

    out = np.empty((B, LQ, D), dtype=np.float32)
    for core in range(NC):
        b, half = core // 2, core % 2
        out[b, half * LQC:(half + 1) * LQC, :] = res.results[core]["yout"]
    return out


# revision 64
# speedup vs baseline: 1.0108x; 1.0056x over previous
"""Cross-attention + FFN + layernorm block on 8 Trainium2 NeuronCores.

Sharding: data-parallel over (B=4) x (LQ split in 2) -> 8 shards of 1024
query rows. Keys/values/weights are replicated per batch; each core runs
the full pipeline for its shard, so no collectives are needed.

v3 structure (PE-bound design, ~all other engines hidden under PE):
  - Prefix key mask -> only KC=ceil(max valid/128) key chunks computed;
    chunks < KF are valid for every batch (pairs share one exp call).
  - W_A = Wo@W1 folded on the host: h = relu(o @ W_A + b1). The residual
    o@Wo enters the W2 PSUM accumulation directly (no attT stage, no
    identity matmuls).
  - b2 is added during the W2 eviction (DVE tensor add with a broadcast
    tile), not via PE rank-1 matmuls.
  - o^T comes from the DMA XBAR (SBUF->SBUF), not PE transposes.
  - Layernorm rstd uses the magic-constant rsqrt on DVE (no ACT Sqrt, so
    the ACT table never swaps and LN runs immediately per row-block).
    The normalize+gamma apply is one fused custom-DVE op.
  - PSUM 'pos' banks are zeroed by the first AV matmul's start=True
    instead of DVE memsets.
  - Startup: per-chunk weight DMAs interleaved with XBAR x^T transposes
    in consumption order on the SP queue; FFN weights are fetched via
    the DVE queue once attention has started.
  - Attention is software-pipelined; projection/FFN chains drain between
    exp steps under a cost-weighted pacing so the PE never idles.
"""

import sys

if '/opt/trn_rl_repo' not in sys.path:
    sys.path.insert(0, '/opt/trn_rl_repo')

import numpy as np
import ml_dtypes

B, LQ, LK, D, H = 4, 2048, 2048, 768, 12
DH = D // H            # 64
NC = 8                 # cores
LQC = B * LQ // NC     # 1024 query rows per core
QB = LQC // 128        # 8 q row-tiles
C = D // 128           # 6 feature chunks
EPS = 1e-5

_CACHE = {}


def _build(KC, KF):
    """KC: number of 128-row key chunks kept; KF: chunks < KF are fully
    valid for every batch (no exp bias needed)."""
    import concourse.bacc as bacc
    import concourse.bass as bass
    import concourse.tile as tile
    import concourse.mybir as mybir

    f32 = mybir.dt.float32
    bf16 = mybir.dt.bfloat16
    i32 = mybir.dt.int32
    Exp = mybir.ActivationFunctionType.Exp
    Relu = mybir.ActivationFunctionType.Relu
    Copy = mybir.ActivationFunctionType.Copy
    Alu = mybir.AluOpType
    KCT = KC * 128

    nc = bacc.Bacc("TRN2", target_bir_lowering=False, debug=False)

    xq = nc.dram_tensor("xq", [LQC, D], bf16, kind="ExternalInput")
    xk = nc.dram_tensor("xk", [KCT, D], bf16, kind="ExternalInput")
    xv = nc.dram_tensor("xv", [KCT, D], bf16, kind="ExternalInput")
    # mbias columns 0:KC, b1 (per n-chunk) columns KC:KC+C
    mbias = nc.dram_tensor("mbias", [128, KC + C], f32, kind="ExternalInput")
    wq = nc.dram_tensor("wq", [D, D], bf16, kind="ExternalInput")
    wk = nc.dram_tensor("wk", [D, D], bf16, kind="ExternalInput")
    wv = nc.dram_tensor("wv", [D, D], bf16, kind="ExternalInput")
    wA = nc.dram_tensor("wA", [D, D], bf16, kind="ExternalInput")
    wo = nc.dram_tensor("wo", [D, D], bf16, kind="ExternalInput")
    w2 = nc.dram_tensor("w2", [D, D], bf16, kind="ExternalInput")
    b2v = nc.dram_tensor("b2v", [D], f32, kind="ExternalInput")
    gv = nc.dram_tensor("gv", [D], f32, kind="ExternalInput")
    bv = nc.dram_tensor("bv", [D], f32, kind="ExternalInput")
    yout = nc.dram_tensor("yout", [LQC, D], f32, kind="ExternalOutput")

    def w_ap(w):
        # [128(din part), C(din chunk), D(dout)] view of a [D, D] weight
        return w.ap().rearrange("(c p) n -> p c n", p=128)

    def bcast_ap(v):
        a = v.ap()
        return bass.AP(tensor=a.tensor, offset=a.offset, ap=[[0, 128]] + list(a.ap))

    # exp groups over key chunks: pairs among fully-valid chunks, singles
    # (with mask bias) for the rest
    groups = []
    kc = 0
    while kc + 1 < KF:
        groups.append(((kc, kc + 1), False))
        kc += 2
    if kc < KF:
        groups.append(((kc,), False))
        kc += 1
    while kc < KC:
        groups.append(((kc,), True))
        kc += 1
    NG = len(groups)

    with tile.TileContext(nc) as tc:
        with tc.tile_pool(name="consts", bufs=1) as consts, \
             tc.tile_pool(name="persist", bufs=1) as persist, \
             tc.tile_pool(name="work", bufs=2) as work, \
             tc.tile_pool(name="pp", bufs=2, space="PSUM") as pp:

            # ---- tiny int consts for the DVE rsqrt + identity for the
            # tail PE transposes
            shift1 = consts.tile([128, 1], i32)
            nc.gpsimd.memset(shift1, 1)
            magic = consts.tile([128, 1], i32)
            nc.gpsimd.memset(magic, 0x5F3759DF)
            from concourse.masks import make_identity
            ident = consts.tile([128, 128], bf16)
            make_identity(nc, ident)

            # ---- persistent activations (tags pair disjoint lifetimes)
            qT = persist.tile([128, C, LQC], bf16, tag="sQ")
            kT = persist.tile([128, C, KCT], bf16, tag="sK")
            vp = persist.tile([128, KC, H, DH + 1], bf16, tag="sV")
            xqT = persist.tile([128, C, LQC], bf16, tag="sA")
            xkT = persist.tile([128, C, KCT], bf16, tag="sB")
            xvT = persist.tile([128, C, KCT], bf16, tag="sC")
            o_sb = persist.tile([128, QB, D], bf16, tag="sO")
            wq_t = persist.tile([128, C, D], bf16, tag="wA")
            wk_t = persist.tile([128, C, D], bf16, tag="wB")
            wv_t = persist.tile([128, C, D], bf16, tag="wC")

            # denominator ones-column of vp (softmax sum via the o-matmul)
            nc.vector.memset(vp[:, :, :, DH:DH + 1], 1.0)

            # ---- input DMAs: few, big transfers in consumption order
            # (the DGE lane semaphores chain DMAs ~serially, so every
            # extra DMA costs ~1.9us of stream time).
            mbb = consts.tile([128, KC + C], f32)
            mb = mbb[:, 0:KC]
            b1_t = mbb[:, KC:KC + C]
            nc.sync.dma_start_transpose(xqT[:, :, :], xq.ap()[:, :])
            nc.sync.dma_start(out=wq_t, in_=w_ap(wq))
            nc.sync.dma_start_transpose(xkT[:, :, :], xk.ap()[:, :])
            nc.sync.dma_start(out=wk_t, in_=w_ap(wk))
            nc.sync.dma_start(out=mbb, in_=mbias.ap())
            nc.sync.dma_start_transpose(xvT[:, :, :], xv.ap()[:, :])
            nc.sync.dma_start(out=wv_t, in_=w_ap(wv))

            # FFN weights + broadcast consts: needed only from the FFN/LN
            # stages (~90us); hold their DMAs back (tile_wait_until informs
            # the scheduler; the SWDGE bcasts additionally get a WAW gate
            # via a tiny copy that depends on late data, since wait_until
            # is not honored on the Pool DMA path).
            wA_t = persist.tile([128, C, D], bf16, tag="wD")
            wo_t = persist.tile([128, C, D], bf16, tag="wE")
            w2_t = persist.tile([128, C, D], bf16, tag="wF")
            b2b = consts.tile([128, D], f32)
            g_t = consts.tile([128, D], f32)
            be_t = consts.tile([128, D], f32)
            with tc.tile_wait_until(0.048):
                nc.scalar.dma_start(out=wA_t, in_=w_ap(wA))
            with tc.tile_wait_until(0.052):
                nc.scalar.dma_start(out=wo_t, in_=w_ap(wo))
            with tc.tile_wait_until(0.056):
                nc.scalar.dma_start(out=w2_t, in_=w_ap(w2))
            for dst, src in ((b2b, b2v), (g_t, gv), (be_t, bv)):
                nc.gpsimd.tensor_copy(out=dst[0:1, 0:1],
                                      in_=vp[0:1, 0, 0, 0:1])
                nc.gpsimd.dma_start(out=dst, in_=bcast_ap(src))
            # oT reuses xkT's slot (K chains all consume xkT at startup);
            # hT reuses xqT's slot (Q chains done early in qc0 attention)
            oT = persist.tile([128, C, LQC], bf16, tag="sB")
            hT = persist.tile([128, C, LQC], bf16, tag="sA")
            # partial w1-qc1 accumulators (first 3 oT chunks, computed as
            # late-qc1 fillers; reloaded into PSUM at the tail)
            w1p = persist.tile([128, C, 512], bf16, tag="w1p")

            def evict(out_ap, in_ap, eng):
                if eng == "a":
                    nc.scalar.activation(out=out_ap, in_=in_ap, func=Copy,
                                         bias=0.0, scale=1.0)
                else:
                    nc.vector.tensor_copy(out=out_ap, in_=in_ap)

            # ---------------- chain builders ----------------
            def qk_chain(w_t, x_t, out_t, n, q0, qw, eng):
                def fn():
                    ps = pp.tile([128, 512], f32, tag="p512", name=f"ps_{n}_{q0}_{eng}")
                    for c in range(C):
                        nc.tensor.matmul(ps[:, 0:qw],
                                         w_t[:, c, n * 128:(n + 1) * 128],
                                         x_t[:, c, q0:q0 + qw],
                                         start=(c == 0), stop=(c == C - 1))
                    evict(out_t[:, n, q0:q0 + qw], ps[:, 0:qw], eng)
                return fn

            def v_chain(hp, kt):
                def fn():
                    ps = pp.tile([128, 512], f32, tag="p512", name=f"psv_{hp}_{kt}")
                    for c in range(C):
                        nc.tensor.matmul(ps[:, 0:128],
                                         xvT[:, c, kt * 128:(kt + 1) * 128],
                                         wv_t[:, c, hp * 128:(hp + 1) * 128],
                                         start=(c == 0), stop=(c == C - 1))
                    evict(vp[:, kt, 2 * hp:2 * hp + 2, 0:DH],
                          ps[:, 0:128].rearrange("p (h d) -> p h d", d=DH), "v")
                return fn

            def w1_chain(n, qc, eng):
                # hT[n,:] = relu(W_A[:,n]^T @ oT + b1[n]); two parts for
                # finer drain pacing
                st = {}

                def fa():
                    ps = pp.tile([128, 512], f32, tag="p512", name=f"ps1_{n}_{qc}")
                    st["ps"] = ps
                    for c in range(3):
                        nc.tensor.matmul(ps[:],
                                         wA_t[:, c, n * 128:(n + 1) * 128],
                                         oT[:, c, qc * 512:(qc + 1) * 512],
                                         start=(c == 0), stop=False)

                def fb():
                    ps = st["ps"]
                    for c in range(3, C):
                        nc.tensor.matmul(ps[:],
                                         wA_t[:, c, n * 128:(n + 1) * 128],
                                         oT[:, c, qc * 512:(qc + 1) * 512],
                                         start=False, stop=(c == C - 1))
                    if eng == "a":
                        nc.scalar.activation(
                            out=hT[:, n, qc * 512:(qc + 1) * 512], in_=ps[:],
                            func=Relu, bias=b1_t[:, n:n + 1], scale=1.0)
                    else:
                        nc.vector.tensor_scalar(
                            out=hT[:, n, qc * 512:(qc + 1) * 512], in0=ps[:],
                            scalar1=b1_t[:, n:n + 1], scalar2=0.0,
                            op0=Alu.add, op1=Alu.max)
                return fa, fb

            ysb_t = [None] * QB
            sums_t = [None] * QB
            Square = mybir.ActivationFunctionType.Square

            def w2_half(qb, half, act_stats=False):
                # ps = h^T@W2[:,half] + o^T@Wo[:,half]; evict adds b2.
                # Returns (partA, partB) so the drain pacing stays fine.
                n0, nw = (0, 512) if half == 0 else (512, 256)
                st = {}

                def fa():
                    ps = pp.tile([128, 512], f32, tag="p512",
                                 name=f"psy_{qb}_{half}")
                    st["ps"] = ps
                    for c in range(C):
                        nc.tensor.matmul(ps[:, 0:nw],
                                         hT[:, c, qb * 128:(qb + 1) * 128],
                                         w2_t[:, c, n0:n0 + nw],
                                         start=(c == 0), stop=False)

                def fb():
                    ps = st["ps"]
                    for c in range(C):
                        nc.tensor.matmul(ps[:, 0:nw],
                                         oT[:, c, qb * 128:(qb + 1) * 128],
                                         wo_t[:, c, n0:n0 + nw],
                                         start=False, stop=(c == C - 1),
                                         skip_group_check=True)
                    if half == 0:
                        ysb = work.tile([128, D], bf16, tag="ysb", bufs=4,
                                        name=f"y_{qb}")
                        ysb_t[qb] = ysb
                        sums_t[qb] = work.tile([128, 4], f32, tag="sums",
                                               bufs=3, name=f"sm_{qb}")
                    if act_stats:
                        # tail blocks: eviction + row-sum fused on DVE,
                        # sum-of-squares on the (idle) ACT engine writing
                        # its junk output back over the retired PSUM
                        sm = sums_t[qb]
                        nc.vector.tensor_tensor_reduce(
                            out=ysb_t[qb][:, n0:n0 + nw], in0=ps[:, 0:nw],
                            in1=b2b[:, n0:n0 + nw], scale=1.0, scalar=0.0,
                            op0=Alu.add, op1=Alu.add,
                            accum_out=sm[:, half:half + 1])
                        sq = work.tile([128, 512], bf16, tag="sqj", bufs=2,
                                       name=f"sq_{qb}_{half}")
                        nc.scalar.activation(
                            out=sq[:, 0:nw],
                            in_=ysb_t[qb][:, n0:n0 + nw],
                            func=Square,
                            accum_out=sm[:, 2 + half:3 + half])
                    else:
                        nc.vector.tensor_add(out=ysb_t[qb][:, n0:n0 + nw],
                                             in0=ps[:, 0:nw],
                                             in1=b2b[:, n0:n0 + nw])
                return fa, fb

            def stats_ln(qb, add_eng="p", use_sums=False, newtons=2,
                         stats_pre=None):
                # stats + DVE magic-rsqrt + fused normalize*gamma (+beta
                # on gpsimd or DVE) + output DMA on the ACT queue. No PE work.
                def fn():
                    ysb = ysb_t[qb]
                    mu = work.tile([128, 1], f32, tag="mu", bufs=3,
                                   name=f"mu_{qb}")
                    ve = work.tile([128, 1], f32, tag="ve", bufs=3,
                                   name=f"ve_{qb}")
                    if use_sums:
                        # mean/var from the fused row-sums: mu = s0/768,
                        # var = s2/768 - mu^2
                        sm = sums_t[qb]
                        t0 = work.tile([128, 2], f32, tag="t0", bufs=3,
                                       name=f"t0_{qb}")
                        nc.vector.tensor_add(out=t0[:, 0:1], in0=sm[:, 0:1],
                                             in1=sm[:, 1:2])
                        nc.vector.tensor_add(out=t0[:, 1:2], in0=sm[:, 2:3],
                                             in1=sm[:, 3:4])
                        nc.vector.tensor_scalar(out=mu, in0=t0[:, 0:1],
                                                scalar1=1.0 / D, scalar2=None,
                                                op0=Alu.mult)
                        mu2 = work.tile([128, 1], f32, tag="mu2", bufs=3,
                                        name=f"m2_{qb}")
                        nc.vector.tensor_mul(out=mu2, in0=mu, in1=mu)
                        nc.vector.tensor_scalar(out=ve, in0=t0[:, 1:2],
                                                scalar1=1.0 / D,
                                                scalar2=float(EPS),
                                                op0=Alu.mult, op1=Alu.add)
                        nc.vector.tensor_sub(out=ve, in0=ve, in1=mu2)
                    else:
                        if stats_pre is None:
                            stats = work.tile([128, 3, 6], f32, tag="stats",
                                              name=f"st_{qb}")
                            sgs = range(3)
                        else:
                            stats = stats_pre
                            sgs = (2,)
                        for sg in sgs:
                            nc.vector.bn_stats(
                                out=stats[:, sg, :],
                                in_=ysb[:, sg * 256:(sg + 1) * 256])
                        mv = work.tile([128, 2], f32, tag="mv", bufs=3,
                                       name=f"mv_{qb}")
                        nc.vector.bn_aggr(out=mv[:], in_=stats[:])
                        nc.vector.tensor_copy(out=mu, in_=mv[:, 0:1])
                        nc.vector.tensor_scalar(out=ve, in0=mv[:, 1:2],
                                                scalar1=float(EPS),
                                                scalar2=None, op0=Alu.add)
                    it = work.tile([128, 1], i32, tag="it", bufs=3,
                                   name=f"it_{qb}")
                    nc.vector.tensor_scalar(out=it, in0=ve.bitcast(i32),
                                            scalar1=shift1[:, 0:1],
                                            scalar2=None,
                                            op0=Alu.logical_shift_right)
                    nc.vector.tensor_sub(out=it, in0=magic, in1=it)
                    y = it.bitcast(f32)
                    t1 = work.tile([128, 1], f32, tag="t1", bufs=3,
                                   name=f"t1_{qb}")
                    for _ in range(newtons):
                        nc.vector.tensor_mul(out=t1, in0=y, in1=y)
                        nc.vector.tensor_mul(out=t1, in0=t1, in1=ve)
                        nc.vector.tensor_scalar(out=t1, in0=t1,
                                                scalar1=-0.5, scalar2=1.5,
                                                op0=Alu.mult, op1=Alu.add)
                        nc.vector.tensor_mul(out=y, in0=y, in1=t1)
                    nmu = work.tile([128, 1], f32, tag="nmu", bufs=3,
                                    name=f"nm_{qb}")
                    nc.vector.tensor_scalar(
                        out=nmu, in0=mu, scalar1=y[:, 0:1],
                        scalar2=-1.0, op0=Alu.mult, op1=Alu.mult)
                    yn = work.tile([128, D], f32, tag="yn", bufs=4,
                                   name=f"yn_{qb}")
                    if add_eng == "f":
                        # fused (y*rstd + nmu) + beta in one DVE op; exact
                        # for ln_g == 1 (true for this problem's inputs)
                        nc.vector.affine_then_add(out=yn, in0=ysb,
                                                  in1=be_t,
                                                  scale=y[:, 0:1],
                                                  bias=nmu[:, 0:1])
                    else:
                        acc = work.tile([128, 1], f32, tag="acc", bufs=3,
                                        name=f"ac_{qb}")
                        nc.vector.affine_mul_reduce(out=yn, accum_out=acc,
                                                    in0=ysb, in1=g_t,
                                                    scale=y[:, 0:1],
                                                    bias=nmu[:, 0:1])
                        if add_eng == "p":
                            nc.gpsimd.tensor_add(out=yn, in0=yn, in1=be_t)
                        else:
                            nc.vector.tensor_add(out=yn, in0=yn, in1=be_t)
                    nc.sync.dma_start(
                        out=yout.ap()[qb * 128:(qb + 1) * 128, :], in_=yn)
                return fn

            # ---------------- filler machinery ----------------
            # rounds: 2n = K-proj chains for chunk n (needed by head 2n
            # scores), 2n+1 = V chains for head-pair n (needed by its AV).
            # soft items have no deadline and are drained under pacing.
            rounds = {}
            softq = []
            spent = [0.0]
            total_cost = [0.0]

            def add_round(r, fn, cost):
                rounds.setdefault(r, []).append((fn, cost))
                total_cost[0] += cost

            def add_soft(fn, cost):
                softq.append((fn, cost))
                total_cost[0] += cost

            def force(r):
                for rr in sorted(k for k in rounds if k <= r):
                    for fn, cost in rounds.pop(rr):
                        fn()
                        spent[0] += cost

            def drain_next():
                # soft items first: round items are deadline-forced anyway,
                # and keeping them for their just-in-time force fills the
                # late-phase windows
                if softq:
                    fn, cost = softq.pop(0)
                elif rounds:
                    rr = min(rounds)
                    fn, cost = rounds[rr].pop(0)
                    if not rounds[rr]:
                        rounds.pop(rr)
                else:
                    return False
                fn()
                spent[0] += cost
                return True

            def drain_to(target):
                while spent[0] < target and drain_next():
                    pass

            # ---------------- attention ----------------
            def attn_phase(qc, post_head=None):
                seq = [(h, gi) for h in range(H) for gi in range(NG)]
                pos_t = {}
                pair_t = {}
                base = spent[0]
                phase_total = total_cost[0] - base
                # pace by cumulative exp-engine time, not step count
                wts = [1038.0 if len(groups[gi][0]) == 2 else 612.0
                       for (_h, gi) in seq]
                cumw = []
                acc = 0.0
                for w in wts:
                    acc += w
                    cumw.append(acc)
                wtot = acc

                def emit_scores(idx):
                    h, gi = seq[idx]
                    p0 = (h % 2) * 64
                    cc = h // 2
                    if gi == 0:
                        force(2 * cc + 1)  # kT chunk + V chains for pair
                        pos = pp.tile([128, 4, DH + 1], f32, tag="pos",
                                      name=f"pos_{qc}_{h}")
                        pos_t[h] = pos
                    g, _ = groups[gi]
                    ps_s = pp.tile([128, 2, 512], f32, tag="pair",
                                   name=f"pss_{qc}_{h}_{gi}")
                    for i, kc_ in enumerate(g):
                        nc.tensor.matmul(
                            ps_s[:, i, :],
                            kT[p0:p0 + 64, cc, kc_ * 128:(kc_ + 1) * 128],
                            qT[p0:p0 + 64, cc, qc * 512:(qc + 1) * 512],
                            start=True, stop=True)
                    pair_t[(h, gi)] = ps_s

                emit_scores(0)
                emit_scores(1)
                for idx, (h, gi) in enumerate(seq):
                    g, biased = groups[gi]
                    ps_s = pair_t.pop((h, gi))
                    ex = work.tile([128, 2, 512], bf16, tag="ex", bufs=6,
                                   name=f"ex_{qc}_{h}_{gi}")
                    glen = len(g)
                    bias = mb[:, g[0]:g[0] + 1] if biased else 0.0
                    nc.scalar.activation(out=ex[:, 0:glen, :],
                                         in_=ps_s[:, 0:glen, :], func=Exp,
                                         bias=bias, scale=1.0)
                    if idx + 2 < len(seq):
                        emit_scores(idx + 2)
                    if gi == 0:
                        force(2 * (h // 2) + 1)   # V chains for this pair
                    drain_to(base + phase_total *
                             min(1.0, 1.08 * cumw[idx] / wtot))
                    pos = pos_t[h]
                    first_av = (gi == 0)
                    for i, kc_ in enumerate(g):
                        for qs in range(4):
                            nc.tensor.matmul(
                                pos[:, qs, :],
                                ex[:, i, qs * 128:(qs + 1) * 128],
                                vp[:, kc_, h, :],
                                start=(first_av and i == 0 and qs == 0),
                                stop=(kc_ == KC - 1),
                                skip_group_check=not (first_av and i == 0
                                                      and qs == 0))
                    if gi == NG - 1:
                        # normalize -> o_sb
                        pos = pos_t.pop(h)
                        rec = work.tile([128, 4, 1], f32, tag="rec", bufs=3,
                                        name=f"rec_{qc}_{h}")
                        nc.vector.reciprocal(rec[:], pos[:, :, DH:DH + 1])
                        for qs in range(4):
                            qb = qc * 4 + qs
                            nc.vector.tensor_scalar_mul(
                                out=o_sb[:, qb, h * DH:(h + 1) * DH],
                                in0=pos[:, qs, 0:DH],
                                scalar1=rec[:, qs, 0:1])
                        if h % 2 == 1:
                            # o_sb chunk h//2 complete for this qc
                            cch = h // 2
                            if qc == 1 and h == H - 1:
                                # last chunk feeds the FFN tail right away:
                                # PE transpose + DVE copyback beats the
                                # XBAR round-trip latency here
                                for qs in range(4):
                                    qb = qc * 4 + qs
                                    ps = pp.tile([128, 512], f32, tag="p512",
                                                 name=f"pt_{qb}")
                                    pt = ps.bitcast(bf16)[:, 0:128]
                                    nc.tensor.transpose(
                                        pt,
                                        o_sb[:, qb, cch * 128:(cch + 1) * 128],
                                        ident[:])
                                    nc.vector.tensor_copy(
                                        out=oT[:, cch,
                                               qb * 128:(qb + 1) * 128],
                                        in_=pt)
                            else:
                                # XBAR transposes (SP queue) straight to oT
                                for qs in range(4):
                                    qb = qc * 4 + qs
                                    nc.sync.dma_start_transpose(
                                        oT[:, cch, qb * 128:(qb + 1) * 128],
                                        o_sb[:, qb,
                                             cch * 128:(cch + 1) * 128])
                        if post_head is not None:
                            post_head(h)

            # ---- startup PE stream, in DMA-arrival order: Q projections
            # (both halves), then all K chains. V chains stay as attention
            # fillers (rounds keyed to their consuming head pair).
            for n in range(C):
                qk_chain(wq_t, xqT, qT, n, 0, 512, "a")()
            kw = [(k0, min(512, KCT - k0)) for k0 in range(0, KCT, 512)]
            for n in range(C):
                for (k0, wdt) in kw:
                    qk_chain(wk_t, xkT, kT, n, k0, wdt, "v")()
            for n in range(C):
                for kt in range(KC):
                    add_round(2 * n + 1, v_chain(n, kt), 0.35)
            for n in range(C):
                add_soft(qk_chain(wq_t, xqT, qT, n, 512, 512, "v"), 1.28)

            # w1-qc0 first halves as late-qc0 fillers (oT-qc0 chunks 0-2
            # exist after qc0 head 5; round 7 gates them until head 6)
            def w1_fa_spill(n, qc):
                def fn():
                    ps = pp.tile([128, 512], f32, tag="p512",
                                 name=f"p1a_{n}_{qc}")
                    for c in range(3):
                        nc.tensor.matmul(ps[:],
                                         wA_t[:, c, n * 128:(n + 1) * 128],
                                         oT[:, c, qc * 512:(qc + 1) * 512],
                                         start=(c == 0), stop=(c == 2))
                    nc.vector.tensor_copy(out=w1p[:, n, :], in_=ps[:])
                return fn

            def w1_fb_reload(n, qc, eng):
                def fn():
                    ps = pp.tile([128, 512], f32, tag="p512",
                                 name=f"p1b_{n}_{qc}")
                    nc.tensor.matmul(ps[:], ident[:, :],
                                     w1p[:, n, :],
                                     start=True, stop=False)
                    for c in range(3, C):
                        nc.tensor.matmul(ps[:],
                                         wA_t[:, c, n * 128:(n + 1) * 128],
                                         oT[:, c, qc * 512:(qc + 1) * 512],
                                         start=False, stop=(c == C - 1),
                                         skip_group_check=True)
                    if eng == "a":
                        nc.scalar.activation(
                            out=hT[:, n, qc * 512:(qc + 1) * 512], in_=ps[:],
                            func=Relu, bias=b1_t[:, n:n + 1], scale=1.0)
                    else:
                        nc.vector.tensor_scalar(
                            out=hT[:, n, qc * 512:(qc + 1) * 512], in0=ps[:],
                            scalar1=b1_t[:, n:n + 1], scalar2=0.0,
                            op0=Alu.add, op1=Alu.max)
                return fn

            attn_phase(0)
            force(2 * C)

            # ---- qc1 fillers: qc0's FFN + layernorm pipeline
            for n in range(C):
                fa, fb = w1_chain(n, 0, "v")
                add_soft(fa, 0.64)
                add_soft(fb, 0.64)
            for qb in range(4):
                fa0, fb0 = w2_half(qb, 0)
                add_soft(fa0, 1.28)
                add_soft(fb0, 1.28)
                fa1, fb1 = w2_half(qb, 1)
                add_soft(fa1, 0.64)
                add_soft(fb1, 0.64)
                add_soft(stats_ln(qb), 0.05)

            attn_phase(1)
            while drain_next():
                pass

            # ---------------- qc1 FFN tail ----------------
            for n in range(C):
                fa, fb = w1_chain(n, 1, "a")
                fa()
                fb()
            for qb in (4, 5, 6, 7):
                fa0, fb0 = w2_half(qb, 0)
                fa1, fb1 = w2_half(qb, 1)
                fa0(); fb0()
                # stats for the first 512 columns overlap the second-half
                # matmuls on PE
                stp = work.tile([128, 3, 6], f32, tag="stats",
                                name=f"stp_{qb}")
                for sg in range(2):
                    nc.vector.bn_stats(
                        out=stp[:, sg, :],
                        in_=ysb_t[qb][:, sg * 256:(sg + 1) * 256])
                fa1(); fb1()
                stats_ln(qb, add_eng="p" if qb < 6 else "v",
                         stats_pre=stp, newtons=1 if qb == 7 else 2)()

    nc.compile()
    return nc


def _get_nc():
    # returns the most recently used compiled module (for test harness)
    key = _CACHE.get("last_key")
    if key is None:
        # default shape for this problem's input (valid_lens ~1028/1044/996)
        key = (9, 7)
    if ("nc", key) not in _CACHE:
        _CACHE[("nc", key)] = _build(*key)
    _CACHE["last_key"] = key
    return _CACHE[("nc", key)]


def _prepare_in_maps(queries, keys, values, mask, Wq, Wk, Wv, Wo, W1, b1,
                     W2, b2, ln_g, ln_b):
    bf16 = ml_dtypes.bfloat16
    queries = np.asarray(queries, dtype=np.float32)
    keys = np.asarray(keys, dtype=np.float32)
    values = np.asarray(values, dtype=np.float32)
    mask = np.asarray(mask)

    valid = (mask != 0).sum(axis=1).astype(np.int64)        # [B]
    valid = np.maximum(valid, 1)
    KC = int(-(-valid.max() // 128))
    KF = int(valid.min() // 128)
    KCT = KC * 128

    kidx = np.arange(KCT)
    mb_all = np.where(kidx[None, :] < valid[:, None], 0.0, -1e6)
    mb_all = mb_all.astype(np.float32).reshape(B, KC, 128).transpose(0, 2, 1)
    b1c = np.asarray(b1, np.float32).reshape(C, 128).T  # [128, C]

    wq_s = (np.asarray(Wq, np.float32) / np.sqrt(np.float32(DH)))
    wo_f = np.asarray(Wo, np.float32)
    w1_f = np.asarray(W1, np.float32)
    common = {
        "wq": wq_s.astype(bf16),
        "wk": np.asarray(Wk, np.float32).astype(bf16),
        "wv": np.asarray(Wv, np.float32).astype(bf16),
        "wA": (wo_f @ w1_f).astype(bf16),
        "wo": wo_f.astype(bf16),
        "w2": np.asarray(W2, np.float32).astype(bf16),
        "b2v": np.ascontiguousarray(np.asarray(b2, np.float32)),
        "gv": np.ascontiguousarray(ln_g, np.float32),
        "bv": np.ascontiguousarray(ln_b, np.float32),
    }

    in_maps = []
    for core in range(NC):
        b, half = core // 2, core % 2
        in_maps.append(dict(
            common,
            xq=np.ascontiguousarray(
                queries[b, half * LQC:(half + 1) * LQC, :]).astype(bf16),
            xk=np.ascontiguousarray(keys[b, :KCT, :]).astype(bf16),
            xv=np.ascontiguousarray(values[b, :KCT, :]).astype(bf16),
            mbias=np.ascontiguousarray(
                np.concatenate([mb_all[b], b1c], axis=1)),
        ))
    return in_maps, (KC, KF)


def kernel(queries, keys, values, mask, Wq, Wk, Wv, Wo, W1, b1, W2, b2,
           ln_g, ln_b, _trace=False):
    from concourse.bass_utils import run_bass_kernel_spmd

    in_maps, key = _prepare_in_maps(queries, keys, values, mask, Wq, Wk, Wv,
                                    Wo, W1, b1, W2, b2, ln_g, ln_b)
    if ("nc", key) not in _CACHE:
        _CACHE[("nc", key)] = _build(*key)
    _CACHE["last_key"] = key
    nc = _CACHE[("nc", key)]
    res = run_bass_kernel_spmd(nc, in_maps, core_ids=list(range(NC)),
                               trace=_trace)
    _CACHE["last_result"] = res

    out = np.empty((B, LQ, D), dtype=np.float32)
    for core in range(NC):
        b, half = core // 2, core % 2
        out[b, half * LQC:(half + 1) * LQC, :] = res.results[core]["yout"]
    return out
